# revision 6
# baseline (speedup 1.0000x reference)
"""Position-attention kernel for Trainium2 (8 NeuronCores, Bass/Tile).

Module: q,k = 1x1 convs to C/8 channels, v = 1x1 conv, attn = softmax(q^T k),
y = v @ attn^T, out = gamma*y + x.  Shapes: B=4, C=512, H=W=64 (N=4096, Cq=64).

Sharding: data-parallel over batch x query-halves -> 8 cores. Core i handles
batch i//2, query positions [h*2048, (h+1)*2048) with h = i%2. Each core
computes full K/V projections for its batch (duplicated across the pair) and
its half of Q, then S^T = k^T q in [key m, query n] layout (no transposes
needed anywhere), exp, and y = v @ attn^T via vT-stationary DoubleRow fp8
matmuls.

Key structure (v2):
- x, weights all fp8e4m3 -> every projection matmul runs DoubleRow (2 c-tiles
  of contraction per pass).  The attention path contributes ~1% of the output
  scale (residual dominates), so fp8 noise there is far below tolerance.
- S^T matmuls have contraction Cq=64 only: q/k are materialized with rows
  duplicated into both partition halves (weight columns duplicated host-side,
  so the projections produce the duplication for free) and each st pair runs
  as TWO concurrent row-tiled matmuls (tile_position (0,0)/(64,0)) -> 2x.
- The softmax denominator is accumulated on the PE by a ones-weights DoubleRow
  matmul per pair into a PSUM bank (all 128 output rows = the denominator, so
  the partition broadcast is free).  This removes the serial DVE add chain and
  the gpsimd partition_all_reduce entirely.
- exp runs once per st PAIR ([128,2,512] in one ACT op) to amortize the ~260ns
  fixed ACT instruction overhead.
- PSUM: u 4 banks + st 1 pair (2 banks) + ones acc 2 bufs (2 banks) = 8.
  st(t+1) is emitted right after exp(t) so the PE's only exposed wait is
  exp(t) - U(t-1) latency (~50ns/iter).

Host-side folds: weights pre-transposed, cast fp8, q/k weight columns (and
biases) duplicated; gamma folded into v_w; gamma*v_b folded into the residual
(softmax rows sum to 1); per-core key permutation puts the core's own query
half first so one SPMD program works for both halves.
"""

import numpy as np
import ml_dtypes

import concourse.bass as bass
import concourse.mybir as mybir
import concourse.tile as tile
from concourse import bacc
from concourse.bass_utils import run_bass_kernel_spmd

BF16 = ml_dtypes.bfloat16
F8NP = ml_dtypes.float8_e4m3

B, C, H, W = 4, 512, 64, 64
N = H * W            # 4096 keys per batch
NQ = N // 2          # 2048 queries per core
CQ = C // 8          # 64 q/k channels
CQ2 = 2 * CQ         # q/k rows duplicated into both partition halves
P = 128
CT = C // P          # 4 channel tiles
CP = CT // 2         # 2 channel-tile pairs (DoubleRow contraction)
MT = N // P          # 32 key tiles
NT = MT // 2         # 16 key-tile pairs
NCH = 512            # matmul moving-dim chunk
QCH = NQ // NCH      # 4 query chunks per core
KCH = N // NCH       # 8 key chunks
NCORES = 8

F32 = mybir.dt.float32
BF = mybir.dt.bfloat16
F8 = mybir.dt.float8e4
F8E = mybir.dt.float8e5
AF = mybir.ActivationFunctionType
DR = mybir.MatmulPerfMode.DoubleRow
LN16 = 2.772588722239781  # exp shift (ln 16): max logit ~10.9 -> e^8.1 ~ 3300 < 57344 (e5m2 max)

_CACHE = {}


def _build_program():
    # Bacc (not raw Bass): its finalize() runs generate_event_semaphores,
    # which splits multi-semaphore waits — walrus codegen allows only one
    # sync wait per instruction.
    nc = bacc.Bacc()

    xb = nc.declare_dram_parameter("xb", [C, N], F8, isOutput=False)
    xr = nc.declare_dram_parameter("xr", [C, NQ], F32, isOutput=False)
    qw = nc.declare_dram_parameter("qw", [C, CQ2], F8, isOutput=False)
    kw = nc.declare_dram_parameter("kw", [C, CQ2], F8, isOutput=False)
    vw = nc.declare_dram_parameter("vw", [C, C], F8, isOutput=False)
    qb = nc.declare_dram_parameter("qb", [CQ2, 1], F32, isOutput=False)
    kb = nc.declare_dram_parameter("kb", [CQ2, 1], F32, isOutput=False)
    out = nc.declare_dram_parameter("out", [C, NQ], F32, isOutput=True)

    with tile.TileContext(nc) as tc:
        with tc.tile_pool(name="consts", bufs=1) as consts:
            x_sb = consts.tile([P, CT * N], F8)        # x[b] as 4 c-tiles side by side
            qw_sb = consts.tile([P, CT * CQ2], F8)
            kw_sb = consts.tile([P, CT * CQ2], F8)
            vw_sb = consts.tile([P, CT * C], F8)
            qb_sb = consts.tile([CQ2, 1], F32)
            kb_sb = consts.tile([CQ2, 1], F32)
            xr_sb = consts.tile([P, CT * NQ], F32)     # residual (+ gamma*v_b) slice
            q_sb = consts.tile([P, NQ], BF)            # rows 0:64 == rows 64:128
            k_sb = consts.tile([P, N], BF)
            vt_sb = consts.tile([P, MT * C], F8)       # vT: 32 m-tiles of [128, 512]
            ones_sb = consts.tile([P, 2 * P], F8)      # DoubleRow ones weights

            # Consolidated input DMAs split across the two HWDGE queues
            # (sync, scalar); x ordered so its first 512 columns land first.
            xb_r = xb[:, :].rearrange("(t p) m -> p t m", p=P)
            xsb_r = x_sb.rearrange("p (t m) -> p t m", t=CT)
            kw_r = kw[:, :].rearrange("(t p) o -> p t o", p=P)
            qw_r = qw[:, :].rearrange("(t p) o -> p t o", p=P)
            vw_r = vw[:, :].rearrange("(t p) o -> p t o", p=P)
            nc.sync.dma_start(out=kw_sb.rearrange("p (t o) -> p t o", t=CT), in_=kw_r)
            nc.scalar.dma_start(out=qw_sb.rearrange("p (t o) -> p t o", t=CT), in_=qw_r)
            nc.scalar.dma_start(out=kb_sb, in_=kb[:, :])
            nc.scalar.dma_start(out=qb_sb, in_=qb[:, :])
            nc.sync.dma_start(out=xsb_r[:, :2, :NCH], in_=xb_r[:, :2, :NCH])
            nc.scalar.dma_start(out=xsb_r[:, 2:, :NCH], in_=xb_r[:, 2:, :NCH])
            nc.scalar.dma_start(out=vw_sb.rearrange("p (t o) -> p t o", t=CT), in_=vw_r)
            nc.sync.dma_start(out=xsb_r[:, :2, NCH:NQ], in_=xb_r[:, :2, NCH:NQ])
            nc.scalar.dma_start(out=xsb_r[:, 2:, NCH:NQ], in_=xb_r[:, 2:, NCH:NQ])
            nc.sync.dma_start(out=xsb_r[:, :2, NQ:], in_=xb_r[:, :2, NQ:])
            nc.scalar.dma_start(out=xsb_r[:, 2:, NQ:], in_=xb_r[:, 2:, NQ:])
            # Touch the bias tiles on ACT before the matmul stream: the
            # Activation-with-bias struct only has one sync-wait slot, so the
            # real bias uses must not need a separate DMA wait.
            bias_touch = consts.tile([CQ2, 2], F32)
            nc.scalar.activation(bias_touch[:, 0:1], kb_sb, AF.Copy)
            nc.scalar.activation(bias_touch[:, 1:2], qb_sb, AF.Copy)
            ln16_sb = consts.tile([P, 1], F32)
            nc.vector.memset(ln16_sb, -LN16)
            nc.vector.memset(ones_sb, 1.0)

            x3 = x_sb.rearrange("p (t m) -> p t m", t=CT)
            kw3 = kw_sb.rearrange("p (t o) -> p t o", t=CT)
            qw3 = qw_sb.rearrange("p (t o) -> p t o", t=CT)
            vw3 = vw_sb.rearrange("p (t o) -> p t o", t=CT)

            # ---- projections (all DoubleRow fp8, 2 c-tile planes per pass) ----
            # Emitted in x-column-arrival order.
            # kq 3 bufs + v 4 bufs = 7 PSUM banks: enough pipelining depth
            # that the MM->drain->reuse latency chain never gates the PE.
            with tc.tile_pool(name="proj_ps", bufs=2, space="PSUM") as proj_ps:
                def k_proj(ch):
                    kp = proj_ps.tile([CQ2, NCH], F32, tag="kq", name="kp", bufs=3)
                    for cp in range(CP):
                        nc.tensor.matmul(
                            kp,
                            lhsT=kw3[:, 2 * cp:2 * cp + 2, :],
                            rhs=x3[:, 2 * cp:2 * cp + 2, ch * NCH:(ch + 1) * NCH],
                            start=(cp == 0), stop=(cp == CP - 1),
                            perf_mode=DR)
                    nc.vector.tensor_scalar_add(
                        k_sb[:, ch * NCH:(ch + 1) * NCH], kp, kb_sb)

                def q_proj(ch):
                    qp = proj_ps.tile([CQ2, NCH], F32, tag="kq", name="qp", bufs=3)
                    for cp in range(CP):
                        nc.tensor.matmul(
                            qp,
                            lhsT=qw3[:, 2 * cp:2 * cp + 2, :],
                            rhs=x3[:, 2 * cp:2 * cp + 2, ch * NCH:(ch + 1) * NCH],
                            start=(cp == 0), stop=(cp == CP - 1),
                            perf_mode=DR)
                    nc.scalar.activation(q_sb[:, ch * NCH:(ch + 1) * NCH], qp,
                                         AF.Identity, bias=qb_sb)

                def v_proj(mt):
                    vp = proj_ps.tile([P, C], F32, tag="v", name="vp", bufs=4)
                    for cp in range(CP):
                        nc.tensor.matmul(
                            vp,
                            lhsT=x3[:, 2 * cp:2 * cp + 2, mt * P:(mt + 1) * P],
                            rhs=vw3[:, 2 * cp:2 * cp + 2, :],
                            start=(cp == 0), stop=(cp == CP - 1),
                            perf_mode=DR)
                    # drains alternate ACT/DVE so neither engine gates the PE
                    if mt % 2 == 0:
                        nc.scalar.activation(vt_sb[:, mt * C:(mt + 1) * C], vp,
                                             AF.Copy)
                    else:
                        nc.vector.tensor_copy(vt_sb[:, mt * C:(mt + 1) * C], vp)

                k_proj(0); q_proj(0)
                for mt in range(4):
                    v_proj(mt)
                for ch in range(1, 4):
                    k_proj(ch); q_proj(ch)
                for mt in range(4, 16):
                    v_proj(mt)
                for ch in range(4, KCH):
                    k_proj(ch)
                for mt in range(16, MT):
                    v_proj(mt)

            nc.scalar.dma_start(out=xr_sb.rearrange("p (t m) -> p t m", t=CT),
                                in_=xr[:, :].rearrange("(t p) m -> p t m", p=P))

            # ---- attention main loop ----
            vt_r = vt_sb.rearrange("p (m c) -> p m c", m=MT)
            ones3 = ones_sb.rearrange("p (j o) -> p j o", j=2)
            out_r = out[:, :].rearrange("(c p) n -> p c n", p=P)
            xr_r = xr_sb.rearrange("p (c m) -> p c m", c=CT)

            with (
                tc.tile_pool(name="u_ps", bufs=1, space="PSUM") as u_ps,
                tc.tile_pool(name="st_ps", bufs=1, space="PSUM") as st_ps,
                tc.tile_pool(name="ones_ps", bufs=2, space="PSUM") as ones_ps,
                tc.tile_pool(name="e_pool", bufs=3) as e_pool,
                tc.tile_pool(name="fin", bufs=2) as fin,
                tc.tile_pool(name="outp", bufs=2) as outp,
            ):
                def emit_st(ch, t):
                    # two concurrent row-tiled matmuls: key tile 2t via
                    # partition rows 0:64, key tile 2t+1 via rows 64:128.
                    st = st_ps.tile([P, 2, NCH], F32, tag="st", name="st")
                    qs0 = q_sb[0:CQ, ch * NCH:(ch + 1) * NCH]
                    qs1 = q_sb[CQ:CQ2, ch * NCH:(ch + 1) * NCH]
                    nc.tensor.matmul(
                        st[:, 0, :], lhsT=k_sb[0:CQ, (2 * t) * P:(2 * t + 1) * P],
                        rhs=qs0, start=True, stop=True)
                    nc.tensor.matmul(
                        st[:, 1, :], lhsT=k_sb[CQ:CQ2, (2 * t + 1) * P:(2 * t + 2) * P],
                        rhs=qs1, start=True, stop=True)
                    return st

                st_cur = emit_st(0, 0)
                for ch in range(QCH):
                    u = u_ps.tile([P, CT * NCH], F32, tag="u", name="u")
                    oacc = ones_ps.tile([P, NCH], F32, tag="oacc", name="oacc")

                    for t in range(NT):
                        e2 = e_pool.tile([P, 2, NCH], F8E, tag="e", name="e2")
                        nc.scalar.activation(e2, st_cur, AF.Exp, bias=ln16_sb)
                        if not (ch == QCH - 1 and t == NT - 1):
                            if t < NT - 1:
                                st_cur = emit_st(ch, t + 1)
                            else:
                                st_cur = emit_st(ch + 1, 0)
                        # ones first: at t=NT-1 the denominator is ready 4 MMs
                        # earlier, shortening the epilogue chain.
                        nc.tensor.matmul(oacc, lhsT=ones3, rhs=e2,
                                         start=(t == 0), stop=(t == NT - 1),
                                         perf_mode=DR)
                        for c in range(CT):
                            nc.tensor.matmul(
                                u[:, c * NCH:(c + 1) * NCH],
                                lhsT=vt_r[:, 2 * t:2 * t + 2, c * P:(c + 1) * P],
                                rhs=e2,
                                start=(t == 0), stop=(t == NT - 1),
                                perf_mode=DR)

                    last = ch == QCH - 1
                    o = outp.tile([P, CT * NCH], F32, tag="o", name="o")
                    o3 = o.rearrange("p (c n) -> p c n", c=CT)
                    xr3 = xr_r[:, :, ch * NCH:(ch + 1) * NCH]
                    if not last:
                        # Drain U out of PSUM on DVE (per c-tile, so the next
                        # chunk's first U matmul only waits for its own tile).
                        uc = outp.tile([P, CT * NCH], F32, tag="uc", name="uc")
                        for c in range(CT):
                            nc.vector.tensor_copy(uc[:, c * NCH:(c + 1) * NCH],
                                                  u[:, c * NCH:(c + 1) * NCH])
                        u3 = uc.rearrange("p (c n) -> p c n", c=CT)
                    else:
                        # Final chunk: DVE reads U straight from PSUM.
                        u3 = u.rearrange("p (c n) -> p c n", c=CT)
                    rec = fin.tile([P, NCH], F32, tag="rec", name="rec")
                    nc.vector.reciprocal_approx_fast(out=rec, in_=oacc)
                    # per c-tile epilogue: no single DVE op exceeds ~0.7us, so
                    # nothing big can sit in the queue ahead of the drains the
                    # next chunk's matmuls wait on, and the out DMA starts
                    # after the first tile instead of after the whole chunk.
                    for c in range(CT):
                        nc.vector.tensor_mul(o3[:, c, :], u3[:, c, :], rec)
                        nc.vector.tensor_add(o3[:, c, :], o3[:, c, :],
                                             xr3[:, c, :])
                        nc.sync.dma_start(
                            out=out_r[:, c, ch * NCH:(ch + 1) * NCH],
                            in_=o3[:, c, :])
    nc.finalize()
    return nc


def _get_program():
    if "nc" not in _CACHE:
        _CACHE["nc"] = _build_program()
    return _CACHE["nc"]


def make_in_maps(x, q_w, q_b, k_w, k_b, v_w, v_b, gamma):
    x = np.asarray(x, dtype=np.float32)
    gamma_f = float(np.asarray(gamma).reshape(-1)[0])
    qwT = np.ascontiguousarray(np.asarray(q_w, np.float32).T)      # [C, CQ]
    kwT = np.ascontiguousarray(np.asarray(k_w, np.float32).T)
    qw2 = np.concatenate([qwT, qwT], axis=1).astype(F8NP)          # [C, 2*CQ]
    kw2 = np.concatenate([kwT, kwT], axis=1).astype(F8NP)
    vwT = np.ascontiguousarray(
        gamma_f * np.asarray(v_w, np.float32).T).astype(F8NP)      # [C, C]
    qb1 = np.asarray(q_b, np.float32).reshape(CQ)
    kb1 = np.asarray(k_b, np.float32).reshape(CQ)
    qb2 = np.concatenate([qb1, qb1]).reshape(CQ2, 1)
    kb2 = np.concatenate([kb1, kb1]).reshape(CQ2, 1)
    gvb = (gamma_f * np.asarray(v_b, np.float32)).reshape(C, 1)

    xf = x.reshape(B, C, N)
    in_maps = []
    for core in range(NCORES):
        b, h = core // 2, core % 2
        mine = xf[b, :, h * NQ:(h + 1) * NQ]
        other = xf[b, :, (1 - h) * NQ:(2 - h) * NQ]
        x_perm = np.concatenate([mine, other], axis=1)
        in_maps.append({
            "xb": x_perm.astype(F8NP),
            "xr": np.ascontiguousarray(mine) + gvb,
            "qw": qw2, "kw": kw2, "vw": vwT,
            "qb": qb2, "kb": kb2,
        })
    return in_maps


def run(in_maps, **kwargs):
    nc = _get_program()
    return run_bass_kernel_spmd(nc, in_maps, list(range(NCORES)), **kwargs)


def gather(results):
    out = np.empty((B, C, N), dtype=np.float32)
    for core in range(NCORES):
        b, h = core // 2, core % 2
        out[b, :, h * NQ:(h + 1) * NQ] = results[core]["out"]
    return out.reshape(B, C, H, W)


def kernel(x, q_w, q_b, k_w, k_b, v_w, v_b, gamma, **_):
    in_maps = make_in_maps(x, q_w, q_b, k_w, k_b, v_w, v_b, gamma)
    res = run(in_maps)
    return gather(res.results)


# revision 9
# speedup vs baseline: 1.0040x; 1.0040x over previous
"""Position-attention kernel for Trainium2 (8 NeuronCores, Bass/Tile).

Module: q,k = 1x1 convs to C/8 channels, v = 1x1 conv, attn = softmax(q^T k),
y = v @ attn^T, out = gamma*y + x.  Shapes: B=4, C=512, H=W=64 (N=4096, Cq=64).

Sharding: data-parallel over batch x query-halves -> 8 cores. Core i handles
batch i//2, query positions [h*2048, (h+1)*2048) with h = i%2. Each core
computes full K/V projections for its batch (duplicated across the pair) and
its half of Q, then S^T = k^T q in [key m, query n] layout (no transposes
needed anywhere), exp, and y = v @ attn^T via vT-stationary DoubleRow fp8
matmuls.

Key structure (v2):
- x, weights all fp8e4m3 -> every projection matmul runs DoubleRow (2 c-tiles
  of contraction per pass).  The attention path contributes ~1% of the output
  scale (residual dominates), so fp8 noise there is far below tolerance.
- S^T matmuls have contraction Cq=64 only: q/k are materialized with rows
  duplicated into both partition halves (weight columns duplicated host-side,
  so the projections produce the duplication for free) and each st pair runs
  as TWO concurrent row-tiled matmuls (tile_position (0,0)/(64,0)) -> 2x.
- The softmax denominator is accumulated on the PE by a ones-weights DoubleRow
  matmul per pair into a PSUM bank (all 128 output rows = the denominator, so
  the partition broadcast is free).  This removes the serial DVE add chain and
  the gpsimd partition_all_reduce entirely.
- exp runs once per st PAIR ([128,2,512] in one ACT op) to amortize the ~260ns
  fixed ACT instruction overhead.
- PSUM: u 4 banks + st 1 pair (2 banks) + ones acc 2 bufs (2 banks) = 8.
  st(t+1) is emitted right after exp(t) so the PE's only exposed wait is
  exp(t) - U(t-1) latency (~50ns/iter).

Host-side folds: weights pre-transposed, cast fp8, q/k weight columns (and
biases) duplicated; gamma folded into v_w; gamma*v_b folded into the residual
(softmax rows sum to 1); per-core key permutation puts the core's own query
half first so one SPMD program works for both halves.
"""

import numpy as np
import ml_dtypes

import concourse.bass as bass
import concourse.mybir as mybir
import concourse.tile as tile
from concourse import bacc
from concourse.bass_utils import run_bass_kernel_spmd

BF16 = ml_dtypes.bfloat16
F8NP = ml_dtypes.float8_e4m3

B, C, H, W = 4, 512, 64, 64
N = H * W            # 4096 keys per batch
NQ = N // 2          # 2048 queries per core
CQ = C // 8          # 64 q/k channels
CQ2 = 2 * CQ         # q/k rows duplicated into both partition halves
P = 128
CT = C // P          # 4 channel tiles
CP = CT // 2         # 2 channel-tile pairs (DoubleRow contraction)
MT = N // P          # 32 key tiles
NT = MT // 2         # 16 key-tile pairs
NCH = 512            # matmul moving-dim chunk
QCH = NQ // NCH      # 4 query chunks per core
KCH = N // NCH       # 8 key chunks
NCORES = 8

F32 = mybir.dt.float32
BF = mybir.dt.bfloat16
F8 = mybir.dt.float8e4
F8E = mybir.dt.float8e5
AF = mybir.ActivationFunctionType
DR = mybir.MatmulPerfMode.DoubleRow
LN16 = 2.772588722239781  # exp shift (ln 16): max logit ~10.9 -> e^8.1 ~ 3300 < 57344 (e5m2 max)

_CACHE = {}


def _build_program():
    # Bacc (not raw Bass): its finalize() runs generate_event_semaphores,
    # which splits multi-semaphore waits — walrus codegen allows only one
    # sync wait per instruction.
    nc = bacc.Bacc()

    xb = nc.declare_dram_parameter("xb", [C, N], F8, isOutput=False)
    xr = nc.declare_dram_parameter("xr", [C, NQ], F32, isOutput=False)
    qw = nc.declare_dram_parameter("qw", [C, CQ2], F8, isOutput=False)
    kw = nc.declare_dram_parameter("kw", [C, CQ2], F8, isOutput=False)
    vw = nc.declare_dram_parameter("vw", [C, C], F8, isOutput=False)
    qb = nc.declare_dram_parameter("qb", [CQ2, 1], F32, isOutput=False)
    kb = nc.declare_dram_parameter("kb", [CQ2, 1], F32, isOutput=False)
    out = nc.declare_dram_parameter("out", [C, NQ], F32, isOutput=True)

    with tile.TileContext(nc) as tc:
        with tc.tile_pool(name="consts", bufs=1) as consts:
            x_sb = consts.tile([P, CT * N], F8)        # x[b] as 4 c-tiles side by side
            qw_sb = consts.tile([P, CT * CQ2], F8)
            kw_sb = consts.tile([P, CT * CQ2], F8)
            vw_sb = consts.tile([P, CT * C], F8)
            qb_sb = consts.tile([CQ2, 1], F32)
            kb_sb = consts.tile([CQ2, 1], F32)
            xr_sb = consts.tile([P, CT * NQ], F32)     # residual (+ gamma*v_b) slice
            q_sb = consts.tile([P, NQ], BF)            # rows 0:64 == rows 64:128
            k_sb = consts.tile([P, N], BF)
            vt_sb = consts.tile([P, MT * C], F8)       # vT: 32 m-tiles of [128, 512]
            ones_sb = consts.tile([P, 2 * P], F8)      # DoubleRow ones weights

            # Consolidated input DMAs split across the two HWDGE queues
            # (sync, scalar); x ordered so its first 512 columns land first.
            xb_r = xb[:, :].rearrange("(t p) m -> p t m", p=P)
            xsb_r = x_sb.rearrange("p (t m) -> p t m", t=CT)
            kw_r = kw[:, :].rearrange("(t p) o -> p t o", p=P)
            qw_r = qw[:, :].rearrange("(t p) o -> p t o", p=P)
            vw_r = vw[:, :].rearrange("(t p) o -> p t o", p=P)
            nc.sync.dma_start(out=kw_sb.rearrange("p (t o) -> p t o", t=CT), in_=kw_r)
            nc.scalar.dma_start(out=qw_sb.rearrange("p (t o) -> p t o", t=CT), in_=qw_r)
            nc.scalar.dma_start(out=kb_sb, in_=kb[:, :])
            nc.scalar.dma_start(out=qb_sb, in_=qb[:, :])
            nc.sync.dma_start(out=xsb_r[:, :2, :NCH], in_=xb_r[:, :2, :NCH])
            nc.scalar.dma_start(out=xsb_r[:, 2:, :NCH], in_=xb_r[:, 2:, :NCH])
            nc.scalar.dma_start(out=vw_sb.rearrange("p (t o) -> p t o", t=CT), in_=vw_r)
            nc.sync.dma_start(out=xsb_r[:, :2, NCH:NQ], in_=xb_r[:, :2, NCH:NQ])
            nc.scalar.dma_start(out=xsb_r[:, 2:, NCH:NQ], in_=xb_r[:, 2:, NCH:NQ])
            nc.sync.dma_start(out=xsb_r[:, :2, NQ:], in_=xb_r[:, :2, NQ:])
            nc.scalar.dma_start(out=xsb_r[:, 2:, NQ:], in_=xb_r[:, 2:, NQ:])
            # Touch the bias tiles on ACT before the matmul stream: the
            # Activation-with-bias struct only has one sync-wait slot, so the
            # real bias uses must not need a separate DMA wait.
            bias_touch = consts.tile([CQ2, 2], F32)
            nc.scalar.activation(bias_touch[:, 0:1], kb_sb, AF.Copy)
            nc.scalar.activation(bias_touch[:, 1:2], qb_sb, AF.Copy)
            ln16_sb = consts.tile([P, 1], F32)
            nc.vector.memset(ln16_sb, -LN16)
            nc.vector.memset(ones_sb, 1.0)

            x3 = x_sb.rearrange("p (t m) -> p t m", t=CT)
            kw3 = kw_sb.rearrange("p (t o) -> p t o", t=CT)
            qw3 = qw_sb.rearrange("p (t o) -> p t o", t=CT)
            vw3 = vw_sb.rearrange("p (t o) -> p t o", t=CT)

            # ---- projections (all DoubleRow fp8, 2 c-tile planes per pass) ----
            # Emitted in x-column-arrival order.
            # Projections drain through WIDE [128,1024] ops (one per 2 tiles):
            # the drain engines (ACT/DVE) are the projection-phase bottleneck,
            # and each op pays ~260ns fixed cost, so halving the op count and
            # doubling the width keeps the drains under the PE matmul pace.
            # PSUM: kq 2x2 banks + v 2x2 banks = 8 (proj scope only).
            with tc.tile_pool(name="proj_ps", bufs=2, space="PSUM") as proj_ps:
                def kq_proj(ch2, w3, bias, is_k):
                    # projects TWO query/key chunks [ch2*1024, (ch2+1)*1024)
                    pp = proj_ps.tile([CQ2, 2 * NCH], F32, tag="kq", name="pp")
                    for half in range(2):
                        cols = slice((2 * ch2 + half) * NCH,
                                     (2 * ch2 + half + 1) * NCH)
                        for cp in range(CP):
                            nc.tensor.matmul(
                                pp[:, half * NCH:(half + 1) * NCH],
                                lhsT=w3[:, 2 * cp:2 * cp + 2, :],
                                rhs=x3[:, 2 * cp:2 * cp + 2, cols],
                                start=(cp == 0), stop=(cp == CP - 1),
                                perf_mode=DR)
                    dst = (k_sb if is_k else q_sb)[:, 2 * ch2 * NCH:
                                                  2 * (ch2 + 1) * NCH]
                    if is_k:
                        nc.vector.tensor_scalar_add(dst, pp, bias)
                    else:
                        nc.scalar.activation(dst, pp, AF.Identity, bias=bias)

                def v_proj2(mp):
                    # projects key tiles 2*mp and 2*mp+1, one wide drain
                    vp = proj_ps.tile([P, 2, C], F32, tag="v", name="vp")
                    for j in range(2):
                        mt = 2 * mp + j
                        for cp in range(CP):
                            nc.tensor.matmul(
                                vp[:, j, :],
                                lhsT=x3[:, 2 * cp:2 * cp + 2, mt * P:(mt + 1) * P],
                                rhs=vw3[:, 2 * cp:2 * cp + 2, :],
                                start=(cp == 0), stop=(cp == CP - 1),
                                perf_mode=DR)
                    # drains alternate ACT/DVE so neither engine gates the PE
                    if mp % 2 == 0:
                        nc.scalar.activation(
                            vt_sb[:, 2 * mp * C:2 * (mp + 1) * C],
                            vp.rearrange("p j o -> p (j o)"), AF.Copy)
                    else:
                        nc.vector.tensor_copy(
                            vt_sb[:, 2 * mp * C:2 * (mp + 1) * C],
                            vp.rearrange("p j o -> p (j o)"))

                kq_proj(0, kw3, kb_sb, True)
                kq_proj(0, qw3, qb_sb, False)
                for mp in range(4):
                    v_proj2(mp)
                kq_proj(1, kw3, kb_sb, True)
                kq_proj(1, qw3, qb_sb, False)
                for mp in range(4, 8):
                    v_proj2(mp)
                for ch2 in range(2, 4):
                    kq_proj(ch2, kw3, kb_sb, True)
                for mp in range(8, MT // 2):
                    v_proj2(mp)

            nc.scalar.dma_start(out=xr_sb.rearrange("p (t m) -> p t m", t=CT),
                                in_=xr[:, :].rearrange("(t p) m -> p t m", p=P))

            # ---- attention main loop ----
            vt_r = vt_sb.rearrange("p (m c) -> p m c", m=MT)
            ones3 = ones_sb.rearrange("p (j o) -> p j o", j=2)
            out_r = out[:, :].rearrange("(c p) n -> p c n", p=P)
            xr_r = xr_sb.rearrange("p (c m) -> p c m", c=CT)

            with (
                tc.tile_pool(name="u_ps", bufs=1, space="PSUM") as u_ps,
                tc.tile_pool(name="st_ps", bufs=1, space="PSUM") as st_ps,
                tc.tile_pool(name="ones_ps", bufs=2, space="PSUM") as ones_ps,
                tc.tile_pool(name="e_pool", bufs=6) as e_pool,
                tc.tile_pool(name="fin", bufs=2) as fin,
                tc.tile_pool(name="outp", bufs=2) as outp,
            ):
                def emit_st(ch, t):
                    # two concurrent row-tiled matmuls: key tile 2t via
                    # partition rows 0:64, key tile 2t+1 via rows 64:128.
                    st = st_ps.tile([P, 2, NCH], F32, tag="st", name="st")
                    qs0 = q_sb[0:CQ, ch * NCH:(ch + 1) * NCH]
                    qs1 = q_sb[CQ:CQ2, ch * NCH:(ch + 1) * NCH]
                    nc.tensor.matmul(
                        st[:, 0, :], lhsT=k_sb[0:CQ, (2 * t) * P:(2 * t + 1) * P],
                        rhs=qs0, start=True, stop=True)
                    nc.tensor.matmul(
                        st[:, 1, :], lhsT=k_sb[CQ:CQ2, (2 * t + 1) * P:(2 * t + 2) * P],
                        rhs=qs1, start=True, stop=True)
                    return st

                st_cur = emit_st(0, 0)
                for ch in range(QCH):
                    u = u_ps.tile([P, CT * NCH], F32, tag="u", name="u")
                    oacc = ones_ps.tile([P, NCH], F32, tag="oacc", name="oacc")

                    for t in range(NT):
                        e2 = e_pool.tile([P, 2, NCH], F8E, tag="e", name="e2")
                        nc.scalar.activation(e2, st_cur, AF.Exp, bias=ln16_sb)
                        if not (ch == QCH - 1 and t == NT - 1):
                            if t < NT - 1:
                                st_cur = emit_st(ch, t + 1)
                            else:
                                st_cur = emit_st(ch + 1, 0)
                        # ones first: at t=NT-1 the denominator is ready 4 MMs
                        # earlier, shortening the epilogue chain.
                        nc.tensor.matmul(oacc, lhsT=ones3, rhs=e2,
                                         start=(t == 0), stop=(t == NT - 1),
                                         perf_mode=DR)
                        for c in range(CT):
                            nc.tensor.matmul(
                                u[:, c * NCH:(c + 1) * NCH],
                                lhsT=vt_r[:, 2 * t:2 * t + 2, c * P:(c + 1) * P],
                                rhs=e2,
                                start=(t == 0), stop=(t == NT - 1),
                                perf_mode=DR)

                    last = ch == QCH - 1
                    o = outp.tile([P, CT * NCH], F32, tag="o", name="o")
                    o3 = o.rearrange("p (c n) -> p c n", c=CT)
                    xr3 = xr_r[:, :, ch * NCH:(ch + 1) * NCH]
                    if not last:
                        # Drain U out of PSUM on DVE (per c-tile, so the next
                        # chunk's first U matmul only waits for its own tile).
                        uc = outp.tile([P, CT * NCH], F32, tag="uc", name="uc")
                        nc.vector.tensor_copy(uc, u)
                        u3 = uc.rearrange("p (c n) -> p c n", c=CT)
                    else:
                        # Final chunk: DVE reads U straight from PSUM.
                        u3 = u.rearrange("p (c n) -> p c n", c=CT)
                    rec = fin.tile([P, NCH], F32, tag="rec", name="rec")
                    nc.vector.reciprocal_approx_fast(out=rec, in_=oacc)
                    # per c-tile epilogue: no single DVE op exceeds ~0.7us, so
                    # nothing big can sit in the queue ahead of the drains the
                    # next chunk's matmuls wait on, and the out DMA starts
                    # after the first tile instead of after the whole chunk.
                    for c in range(CT):
                        nc.vector.tensor_mul(o3[:, c, :], u3[:, c, :], rec)
                        nc.vector.tensor_add(o3[:, c, :], o3[:, c, :],
                                             xr3[:, c, :])
                        nc.sync.dma_start(
                            out=out_r[:, c, ch * NCH:(ch + 1) * NCH],
                            in_=o3[:, c, :])
    nc.finalize()
    return nc


def _get_program():
    if "nc" not in _CACHE:
        _CACHE["nc"] = _build_program()
    return _CACHE["nc"]


def make_in_maps(x, q_w, q_b, k_w, k_b, v_w, v_b, gamma):
    x = np.asarray(x, dtype=np.float32)
    gamma_f = float(np.asarray(gamma).reshape(-1)[0])
    qwT = np.ascontiguousarray(np.asarray(q_w, np.float32).T)      # [C, CQ]
    kwT = np.ascontiguousarray(np.asarray(k_w, np.float32).T)
    qw2 = np.concatenate([qwT, qwT], axis=1).astype(F8NP)          # [C, 2*CQ]
    kw2 = np.concatenate([kwT, kwT], axis=1).astype(F8NP)
    vwT = np.ascontiguousarray(
        gamma_f * np.asarray(v_w, np.float32).T).astype(F8NP)      # [C, C]
    qb1 = np.asarray(q_b, np.float32).reshape(CQ)
    kb1 = np.asarray(k_b, np.float32).reshape(CQ)
    qb2 = np.concatenate([qb1, qb1]).reshape(CQ2, 1)
    kb2 = np.concatenate([kb1, kb1]).reshape(CQ2, 1)
    gvb = (gamma_f * np.asarray(v_b, np.float32)).reshape(C, 1)

    xf = x.reshape(B, C, N)
    in_maps = []
    for core in range(NCORES):
        b, h = core // 2, core % 2
        mine = xf[b, :, h * NQ:(h + 1) * NQ]
        other = xf[b, :, (1 - h) * NQ:(2 - h) * NQ]
        x_perm = np.concatenate([mine, other], axis=1)
        in_maps.append({
            "xb": x_perm.astype(F8NP),
            "xr": np.ascontiguousarray(mine) + gvb,
            "qw": qw2, "kw": kw2, "vw": vwT,
            "qb": qb2, "kb": kb2,
        })
    return in_maps


def run(in_maps, **kwargs):
    nc = _get_program()
    return run_bass_kernel_spmd(nc, in_maps, list(range(NCORES)), **kwargs)


def gather(results):
    out = np.empty((B, C, N), dtype=np.float32)
    for core in range(NCORES):
        b, h = core // 2, core % 2
        out[b, :, h * NQ:(h + 1) * NQ] = results[core]["out"]
    return out.reshape(B, C, H, W)


def kernel(x, q_w, q_b, k_w, k_b, v_w, v_b, gamma, **_):
    in_maps = make_in_maps(x, q_w, q_b, k_w, k_b, v_w, v_b, gamma)
    res = run(in_maps)
    return gather(res.results)


# revision 13
# speedup vs baseline: 1.0804x; 1.0760x over previous
"""Position-attention kernel for Trainium2 (8 NeuronCores, Bass/Tile).

Module: q,k = 1x1 convs to C/8 channels, v = 1x1 conv, attn = softmax(q^T k),
y = v @ attn^T, out = gamma*y + x.  Shapes: B=4, C=512, H=W=64 (N=4096, Cq=64).

Sharding: data-parallel over batch x query-halves -> 8 cores. Core i handles
batch i//2, query positions [h*2048, (h+1)*2048) with h = i%2. Each core
computes full K/V projections for its batch (duplicated across the pair) and
its half of Q, then S^T = k^T q in [key m, query n] layout (no transposes
needed anywhere), exp, and y = v @ attn^T via vT-stationary DoubleRow fp8
matmuls.

Key structure (v2):
- x, weights all fp8e4m3 -> every projection matmul runs DoubleRow (2 c-tiles
  of contraction per pass).  The attention path contributes ~1% of the output
  scale (residual dominates), so fp8 noise there is far below tolerance.
- S^T matmuls have contraction Cq=64 only: q/k are materialized with rows
  duplicated into both partition halves (weight columns duplicated host-side,
  so the projections produce the duplication for free) and each st pair runs
  as TWO concurrent row-tiled matmuls (tile_position (0,0)/(64,0)) -> 2x.
- The softmax denominator is accumulated on the PE by a ones-weights DoubleRow
  matmul per pair into a PSUM bank (all 128 output rows = the denominator, so
  the partition broadcast is free).  This removes the serial DVE add chain and
  the gpsimd partition_all_reduce entirely.
- exp runs once per st PAIR ([128,2,512] in one ACT op) to amortize the ~260ns
  fixed ACT instruction overhead.
- PSUM: u 4 banks + st 1 pair (2 banks) + ones acc 2 bufs (2 banks) = 8.
  st(t+1) is emitted right after exp(t) so the PE's only exposed wait is
  exp(t) - U(t-1) latency (~50ns/iter).

Host-side folds: weights pre-transposed, cast fp8, q/k weight columns (and
biases) duplicated; gamma folded into v_w; gamma*v_b folded into the residual
(softmax rows sum to 1); per-core key permutation puts the core's own query
half first so one SPMD program works for both halves.
"""

import numpy as np
import ml_dtypes

import concourse.bass as bass
import concourse.mybir as mybir
import concourse.tile as tile
from concourse import bacc
from concourse.bass_utils import run_bass_kernel_spmd

BF16 = ml_dtypes.bfloat16
F8NP = ml_dtypes.float8_e4m3

B, C, H, W = 4, 512, 64, 64
N = H * W            # 4096 keys per batch
NQ = N // 2          # 2048 queries per core
CQ = C // 8          # 64 q/k channels
CQ2 = 2 * CQ         # q/k rows duplicated into both partition halves
P = 128
CT = C // P          # 4 channel tiles
CP = CT // 2         # 2 channel-tile pairs (DoubleRow contraction)
MT = N // P          # 32 key tiles
NT = MT // 2         # 16 key-tile pairs
NCH = 512            # matmul moving-dim chunk
QCH = NQ // NCH      # 4 query chunks per core
KCH = N // NCH       # 8 key chunks
NCORES = 8

F32 = mybir.dt.float32
BF = mybir.dt.bfloat16
F8 = mybir.dt.float8e4
F8E = mybir.dt.float8e5
AF = mybir.ActivationFunctionType
DR = mybir.MatmulPerfMode.DoubleRow
LN16 = 2.772588722239781  # exp shift (ln 16): max logit ~10.9 -> e^8.1 ~ 3300 < 57344 (e5m2 max)

_CACHE = {}


def _build_program():
    # Bacc (not raw Bass): its finalize() runs generate_event_semaphores,
    # which splits multi-semaphore waits — walrus codegen allows only one
    # sync wait per instruction.
    nc = bacc.Bacc()

    xb = nc.declare_dram_parameter("xb", [C, N], F8, isOutput=False)
    xr = nc.declare_dram_parameter("xr", [C, NQ], F32, isOutput=False)
    qw = nc.declare_dram_parameter("qw", [C, CQ2], F8, isOutput=False)
    kw = nc.declare_dram_parameter("kw", [C, CQ2], F8, isOutput=False)
    vw = nc.declare_dram_parameter("vw", [C, C], F8, isOutput=False)
    qb = nc.declare_dram_parameter("qb", [CQ2, 1], F32, isOutput=False)
    kb = nc.declare_dram_parameter("kb", [CQ2, 1], F32, isOutput=False)
    out = nc.declare_dram_parameter("out", [C, NQ], F32, isOutput=True)

    with tile.TileContext(nc) as tc:
        with tc.tile_pool(name="consts", bufs=1) as consts:
            x_sb = consts.tile([P, CT * N], F8)        # x[b] as 4 c-tiles side by side
            qw_sb = consts.tile([P, CT * CQ2], F8)
            kw_sb = consts.tile([P, CT * CQ2], F8)
            vw_sb = consts.tile([P, CT * C], F8)
            qb_sb = consts.tile([CQ2, 1], F32)
            kb_sb = consts.tile([CQ2, 1], F32)
            xr_sb = consts.tile([P, CT * NQ], F32)     # residual (+ gamma*v_b) slice
            q_sb = consts.tile([P, NQ], BF)            # rows 0:64 == rows 64:128
            k_sb = consts.tile([P, N], BF)
            vt_sb = consts.tile([P, MT * C], F8)       # vT: 32 m-tiles of [128, 512]
            ones_sb = consts.tile([P, 2 * P], F8)      # DoubleRow ones weights

            # Consolidated input DMAs split across the two HWDGE queues
            # (sync, scalar); x ordered so its first 512 columns land first.
            xb_r = xb[:, :].rearrange("(t p) m -> p t m", p=P)
            xsb_r = x_sb.rearrange("p (t m) -> p t m", t=CT)
            kw_r = kw[:, :].rearrange("(t p) o -> p t o", p=P)
            qw_r = qw[:, :].rearrange("(t p) o -> p t o", p=P)
            vw_r = vw[:, :].rearrange("(t p) o -> p t o", p=P)
            nc.sync.dma_start(out=kw_sb.rearrange("p (t o) -> p t o", t=CT), in_=kw_r)
            nc.scalar.dma_start(out=qw_sb.rearrange("p (t o) -> p t o", t=CT), in_=qw_r)
            nc.scalar.dma_start(out=kb_sb, in_=kb[:, :])
            nc.scalar.dma_start(out=qb_sb, in_=qb[:, :])
            nc.sync.dma_start(out=xsb_r[:, :2, :NCH], in_=xb_r[:, :2, :NCH])
            nc.scalar.dma_start(out=xsb_r[:, 2:, :NCH], in_=xb_r[:, 2:, :NCH])
            nc.scalar.dma_start(out=vw_sb.rearrange("p (t o) -> p t o", t=CT), in_=vw_r)
            nc.sync.dma_start(out=xsb_r[:, :2, NCH:NQ], in_=xb_r[:, :2, NCH:NQ])
            nc.scalar.dma_start(out=xsb_r[:, 2:, NCH:NQ], in_=xb_r[:, 2:, NCH:NQ])
            nc.sync.dma_start(out=xsb_r[:, :2, NQ:], in_=xb_r[:, :2, NQ:])
            nc.scalar.dma_start(out=xsb_r[:, 2:, NQ:], in_=xb_r[:, 2:, NQ:])
            # Touch the bias tiles on ACT before the matmul stream: the
            # Activation-with-bias struct only has one sync-wait slot, so the
            # real bias uses must not need a separate DMA wait.
            bias_touch = consts.tile([CQ2, 2], F32)
            nc.scalar.activation(bias_touch[:, 0:1], kb_sb, AF.Copy)
            nc.scalar.activation(bias_touch[:, 1:2], qb_sb, AF.Copy)
            ln16_sb = consts.tile([P, 1], F32)
            nc.vector.memset(ln16_sb, -LN16)
            nc.vector.memset(ones_sb, 1.0)

            x3 = x_sb.rearrange("p (t m) -> p t m", t=CT)
            kw3 = kw_sb.rearrange("p (t o) -> p t o", t=CT)
            qw3 = qw_sb.rearrange("p (t o) -> p t o", t=CT)
            vw3 = vw_sb.rearrange("p (t o) -> p t o", t=CT)

            # ---- projections (all DoubleRow fp8, 2 c-tile planes per pass) ----
            # Emitted in x-column-arrival order.
            # Projections drain through WIDE [128,1024] ops (one per 2 tiles):
            # the drain engines (ACT/DVE) are the projection-phase bottleneck,
            # and each op pays ~260ns fixed cost, so halving the op count and
            # doubling the width keeps the drains under the PE matmul pace.
            # PSUM: kq 2x2 banks + v 2x2 banks = 8 (proj scope only).
            with tc.tile_pool(name="proj_ps", bufs=2, space="PSUM") as proj_ps:
                def kq_proj(ch2, w3, bias, is_k):
                    # projects TWO query/key chunks [ch2*1024, (ch2+1)*1024)
                    pp = proj_ps.tile([CQ2, 2 * NCH], F32, tag="kq", name="pp")
                    for half in range(2):
                        cols = slice((2 * ch2 + half) * NCH,
                                     (2 * ch2 + half + 1) * NCH)
                        for cp in range(CP):
                            nc.tensor.matmul(
                                pp[:, half * NCH:(half + 1) * NCH],
                                lhsT=w3[:, 2 * cp:2 * cp + 2, :],
                                rhs=x3[:, 2 * cp:2 * cp + 2, cols],
                                start=(cp == 0), stop=(cp == CP - 1),
                                perf_mode=DR)
                    dst = (k_sb if is_k else q_sb)[:, 2 * ch2 * NCH:
                                                  2 * (ch2 + 1) * NCH]
                    if is_k:
                        nc.vector.tensor_scalar_add(dst, pp, bias)
                    else:
                        nc.scalar.activation(dst, pp, AF.Identity, bias=bias)

                def v_proj2(mp):
                    # projects key tiles 2*mp and 2*mp+1, one wide drain
                    vp = proj_ps.tile([P, 2, C], F32, tag="v", name="vp")
                    for j in range(2):
                        mt = 2 * mp + j
                        for cp in range(CP):
                            nc.tensor.matmul(
                                vp[:, j, :],
                                lhsT=x3[:, 2 * cp:2 * cp + 2, mt * P:(mt + 1) * P],
                                rhs=vw3[:, 2 * cp:2 * cp + 2, :],
                                start=(cp == 0), stop=(cp == CP - 1),
                                perf_mode=DR)
                    # drains alternate ACT/DVE so neither engine gates the PE
                    if mp % 2 == 0:
                        nc.scalar.activation(
                            vt_sb[:, 2 * mp * C:2 * (mp + 1) * C],
                            vp.rearrange("p j o -> p (j o)"), AF.Copy)
                    else:
                        nc.vector.tensor_copy(
                            vt_sb[:, 2 * mp * C:2 * (mp + 1) * C],
                            vp.rearrange("p j o -> p (j o)"))

                kq_proj(0, kw3, kb_sb, True)
                kq_proj(0, qw3, qb_sb, False)
                for mp in range(4):
                    v_proj2(mp)
                kq_proj(1, kw3, kb_sb, True)
                kq_proj(1, qw3, qb_sb, False)
                for mp in range(4, 8):
                    v_proj2(mp)
                for ch2 in range(2, 4):
                    kq_proj(ch2, kw3, kb_sb, True)
                for mp in range(8, MT // 2):
                    v_proj2(mp)

            nc.scalar.dma_start(out=xr_sb.rearrange("p (t m) -> p t m", t=CT),
                                in_=xr[:, :].rearrange("(t p) m -> p t m", p=P))

            # ---- attention main loop ----
            # Software-pipelined across chunks: chunk ch's st/exp/ones stream
            # runs during its own iterations, while its U matmuls (which only
            # need the persisted e tiles) run as backlog during chunk ch+1's
            # iterations, 4 per iteration, c-tile-outer so u needs only one
            # PSUM bank at a time.  This frees enough banks for TWO st pair
            # buffers, so st(t+1) no longer waits on exp(t) — the loop paces
            # at max(PE throughput, ACT exp rate) instead of the serial
            # st->exp->st latency chain.
            # PSUM: st 2x2 + u 2x1 + ones 2x1 = 8 banks.
            vt_r = vt_sb.rearrange("p (m c) -> p m c", m=MT)
            ones3 = ones_sb.rearrange("p (j o) -> p j o", j=2)
            out_r = out[:, :].rearrange("(c p) n -> p c n", p=P)
            xr_r = xr_sb.rearrange("p (c m) -> p c m", c=CT)

            with (
                tc.tile_pool(name="u_ps", bufs=2, space="PSUM") as u_ps,
                tc.tile_pool(name="st_ps", bufs=2, space="PSUM") as st_ps,
                tc.tile_pool(name="ones_ps", bufs=2, space="PSUM") as ones_ps,
                tc.tile_pool(name="e_pool", bufs=3) as e_pool,
                tc.tile_pool(name="fin", bufs=2) as fin,
                tc.tile_pool(name="outp", bufs=4) as outp,
            ):
                def emit_st(ch, t):
                    # two concurrent row-tiled matmuls: key tile 2t via
                    # partition rows 0:64, key tile 2t+1 via rows 64:128.
                    st = st_ps.tile([P, 2, NCH], F32, tag="st", name="st")
                    qs0 = q_sb[0:CQ, ch * NCH:(ch + 1) * NCH]
                    qs1 = q_sb[CQ:CQ2, ch * NCH:(ch + 1) * NCH]
                    nc.tensor.matmul(
                        st[:, 0, :], lhsT=k_sb[0:CQ, (2 * t) * P:(2 * t + 1) * P],
                        rhs=qs0, start=True, stop=True)
                    nc.tensor.matmul(
                        st[:, 1, :], lhsT=k_sb[CQ:CQ2, (2 * t + 1) * P:(2 * t + 2) * P],
                        rhs=qs1, start=True, stop=True)
                    return st

                def emit_u_group(ch, c, j, e4, u_c, rec):
                    # 4 U matmuls of chunk ch: c-tile c, key-tile-pairs
                    # 4j..4j+3; after j==3 the c-tile is complete -> epilogue.
                    for tp in range(4 * j, 4 * j + 4):
                        nc.tensor.matmul(
                            u_c,
                            lhsT=vt_r[:, 2 * tp:2 * tp + 2, c * P:(c + 1) * P],
                            rhs=e4[:, tp, :, :],
                            start=(tp == 0), stop=(tp == NT - 1),
                            perf_mode=DR)
                    if j == 3:
                        # c-tile complete: o_c = u_c * rec + xr, DMA out.
                        # DVE reads u straight from PSUM; the mul is also the
                        # drain that frees the u bank for c+2.
                        o = outp.tile([P, NCH], F32, tag="o", name="o")
                        nc.vector.tensor_mul(o, u_c, rec)
                        nc.vector.tensor_add(o, o, xr_r[:, c, ch * NCH:(ch + 1) * NCH])
                        nc.sync.dma_start(
                            out=out_r[:, c, ch * NCH:(ch + 1) * NCH], in_=o)

                # per-chunk state
                e_tiles = [None] * QCH
                oaccs = [None] * QCH
                recs = [None] * QCH

                def iteration(g):
                    """One global iteration: exp+st of chunk ch=g//NT at
                    t=g%NT, trailing ones of g-1, U backlog of chunk ch-1."""
                    ch, t = divmod(g, NT)
                    NG = QCH * NT
                    # exp for (ch, t)
                    if g < NG:
                        if t == 0:
                            e_tiles[ch] = e_pool.tile([P, NT, 2, NCH], F8E,
                                                      tag="e", name="e4")
                            oaccs[ch] = ones_ps.tile([P, NCH], F32,
                                                     tag="oacc", name="oacc")
                        nc.scalar.activation(e_tiles[ch][:, t, :, :],
                                             state["st"][g % 2], AF.Exp,
                                             bias=ln16_sb)
                        if g + 1 < NG:
                            nch, nt = divmod(g + 1, NT)
                            state["st"][(g + 1) % 2] = emit_st(nch, nt)
                    # trailing ones matmul for iteration g-1's exp output
                    if 0 <= g - 1 < NG:
                        pch, pt = divmod(g - 1, NT)
                        nc.tensor.matmul(oaccs[pch],
                                         lhsT=ones3,
                                         rhs=e_tiles[pch][:, pt, :, :],
                                         start=(pt == 0), stop=(pt == NT - 1),
                                         perf_mode=DR)
                        if pt == NT - 1:
                            rec = fin.tile([P, NCH], F32, tag="rec", name="rec")
                            nc.vector.reciprocal_approx_fast(out=rec,
                                                             in_=oaccs[pch])
                            recs[pch] = rec
                    # U backlog of chunk ch-1 (or the final chunk's tail)
                    bch, bt = divmod(g - NT, NT)
                    if g >= NT and bch < QCH:
                        c, j = divmod(bt, 4)
                        if j == 0:
                            state["u"][c % 2] = u_ps.tile([P, NCH], F32,
                                                          tag="u", name="u")
                        emit_u_group(bch, c, j, e_tiles[bch],
                                     state["u"][c % 2], recs[bch])

                state = {"st": [None, None], "u": [None, None]}
                state["st"][0] = emit_st(0, 0)
                for g in range(QCH * NT + NT):
                    iteration(g)
    nc.finalize()
    return nc


def _get_program():
    if "nc" not in _CACHE:
        _CACHE["nc"] = _build_program()
    return _CACHE["nc"]


def make_in_maps(x, q_w, q_b, k_w, k_b, v_w, v_b, gamma):
    x = np.asarray(x, dtype=np.float32)
    gamma_f = float(np.asarray(gamma).reshape(-1)[0])
    qwT = np.ascontiguousarray(np.asarray(q_w, np.float32).T)      # [C, CQ]
    kwT = np.ascontiguousarray(np.asarray(k_w, np.float32).T)
    qw2 = np.concatenate([qwT, qwT], axis=1).astype(F8NP)          # [C, 2*CQ]
    kw2 = np.concatenate([kwT, kwT], axis=1).astype(F8NP)
    vwT = np.ascontiguousarray(
        gamma_f * np.asarray(v_w, np.float32).T).astype(F8NP)      # [C, C]
    qb1 = np.asarray(q_b, np.float32).reshape(CQ)
    kb1 = np.asarray(k_b, np.float32).reshape(CQ)
    qb2 = np.concatenate([qb1, qb1]).reshape(CQ2, 1)
    kb2 = np.concatenate([kb1, kb1]).reshape(CQ2, 1)
    gvb = (gamma_f * np.asarray(v_b, np.float32)).reshape(C, 1)

    xf = x.reshape(B, C, N)
    in_maps = []
    for core in range(NCORES):
        b, h = core // 2, core % 2
        mine = xf[b, :, h * NQ:(h + 1) * NQ]
        other = xf[b, :, (1 - h) * NQ:(2 - h) * NQ]
        x_perm = np.concatenate([mine, other], axis=1)
        in_maps.append({
            "xb": x_perm.astype(F8NP),
            "xr": np.ascontiguousarray(mine) + gvb,
            "qw": qw2, "kw": kw2, "vw": vwT,
            "qb": qb2, "kb": kb2,
        })
    return in_maps


def run(in_maps, **kwargs):
    nc = _get_program()
    return run_bass_kernel_spmd(nc, in_maps, list(range(NCORES)), **kwargs)


def gather(results):
    out = np.empty((B, C, N), dtype=np.float32)
    for core in range(NCORES):
        b, h = core // 2, core % 2
        out[b, :, h * NQ:(h + 1) * NQ] = results[core]["out"]
    return out.reshape(B, C, H, W)


def kernel(x, q_w, q_b, k_w, k_b, v_w, v_b, gamma, **_):
    in_maps = make_in_maps(x, q_w, q_b, k_w, k_b, v_w, v_b, gamma)
    res = run(in_maps)
    return gather(res.results)
